# revision 10
# baseline (speedup 1.0000x reference)
"""Trainium2 Bass kernel for nn_BiLSTM_M_61615600828569 (segment_reduce).

Full computation per batch:
  span_emb = masked-max-pool of token windows   (B,256,768)
  vertex_emb = masked-mean over coref spans     (B,128,768)
  head/tail  = vertex gather by relation        (B,512,768)
  feat = [head, eh, tail, et, head*tail]        (B,512,2344)
  out  = relu(feat @ W1) @ W2 + b2              (B,512,97)

Sharding: data-parallel over batch; 16 batches / 8 cores = 2 per core.
All index work (gather tables, one-hot select matrices, pooling weights)
is precomputed on host; all float math runs on device in bf16 with fp32
PSUM accumulation, in transposed layout (features on partitions) so the
final predict.T has the 97 classes on partitions for a per-partition
bias add.

Span pooling: token rows are fetched with dma_gather at QUAD granularity
(elem = 4 overlapping rows via elem_step) — pass 0 reads rows start..start+3
(rows past the width are killed by one broadcast additive -2e30 mask on the
DVE); pass 1 reads start+min(4, w-3).. which with pass 0 covers
[start, start+w] for w>=3, and is redirected to a staged NEG quad for w<3.
Passes are split per batch so batch-0 compute starts while batch-1 still
gathers.
W1 is zero-padded to 20 uniform 128-row contraction chunks so the eh/et
blocks ride the same accumulation loop (their rhs rows past row 19 are
zeros times zero weights).
"""
import numpy as np
import ml_dtypes
from contextlib import ExitStack

import concourse.bass as bass
import concourse.bacc as bacc
import concourse.tile as tile
from concourse import mybir
from concourse import bass_utils

BF16 = ml_dtypes.bfloat16

B, S, D = 16, 1024, 768
NS, MAXW = 256, 8
V, C = 128, 6
R = 512
REL, HID, DIS = 97, 384, 20
NEG = -1e30

NCORES = 8
NB = B // NCORES          # batches per core = 2
GS = NB * NS              # spans per core = 512
NQ = GS // 128            # span groups = 4
NPASS = 2                 # quad passes per batch
SENT_ROWS = NB * S + 4    # staged sentence rows + four NEG rows (NEG quad)
NEGROW = NB * S
NKC = 20                  # uniform 128-row W1 contraction chunks
W1PAD = NKC * 128

FEAT_BLOCKS = [(0, 768), (768, 788), (788, 1556), (1556, 1576), (1576, 2344)]


def _patch_drain_and_barrier():
    """Walrus rejects >1 explicit sync wait on a Drain (TPB_CTRL), but Tile's
    tail drain waits on every used proc sem at once. Emit one single-wait
    drain per proc instead; the final drain then needs no waits."""
    import concourse.tile as tile_mod
    from concourse.vector_clock import VectorClock, ScopedClock

    if getattr(tile_mod.TileContext, "_ant_drain_patched", False):
        return

    def _patched(self, tick_clock, wait_clock):
        full = tick_clock.global_clock
        n = len(full)
        engines = [self.nc.sync, self.nc.vector, self.nc.scalar,
                   self.nc.tensor, self.nc.gpsimd]
        for i, p in enumerate([q for q in range(n) if full[q] > 0]):
            vec = [full[q] if q == p else 0 for q in range(n)]
            d = engines[i % len(engines)].drain()
            wait_clock.add_sem_waits(d.ins, ScopedClock({None: VectorClock(vec)}))
        self.nc.sync.drain()
        self.nc.all_engine_barrier()
        popped = self.nc._tile_sem_poison_stack.pop()
        assert popped is self._sem_poison
        self.nc.clear_and_free_semaphores(list(self.sems.allocated().values()))
        self.nc.all_engine_barrier()

    tile_mod.TileContext._drain_and_barrier = _patched
    tile_mod.TileContext._ant_drain_patched = True


_patch_drain_and_barrier()

_NC_CACHE = None


def _build():
    """One-core program; SPMD-replicated across the 8 cores."""
    bf = mybir.dt.bfloat16
    f32 = mybir.dt.float32
    AF = mybir.ActivationFunctionType
    MAX = mybir.AluOpType.max

    nc = bacc.Bacc("TRN2", target_bir_lowering=False, debug=False, num_devices=1)

    sent = nc.dram_tensor("sent", (SENT_ROWS, D), bf, kind="ExternalInput")
    gidx = nc.dram_tensor("gidx", (128, NB, NPASS, 16), mybir.dt.int16, kind="ExternalInput")
    w0m = nc.dram_tensor("w0m", (128, NQ, 3), bf, kind="ExternalInput")
    poolt = nc.dram_tensor("poolt", (128, NB, 2, V), bf, kind="ExternalInput")
    invcnt = nc.dram_tensor("invcnt", (V, NB), f32, kind="ExternalInput")
    hsel = nc.dram_tensor("hsel", (V, NB, R), bf, kind="ExternalInput")
    tsel = nc.dram_tensor("tsel", (V, NB, R), bf, kind="ExternalInput")
    eht = nc.dram_tensor("eht", (DIS, NB, R), bf, kind="ExternalInput")
    ett = nc.dram_tensor("ett", (DIS, NB, R), bf, kind="ExternalInput")
    w1 = nc.dram_tensor("w1", (128, NKC, HID), bf, kind="ExternalInput")
    w2 = nc.dram_tensor("w2", (128, HID // 128, REL), bf, kind="ExternalInput")
    b2t = nc.dram_tensor("b2t", (REL, 1), f32, kind="ExternalInput")
    outd = nc.dram_tensor("outd", (NB, REL, R), f32, kind="ExternalOutput")

    # overlapping-quad view of the staged sentence: row i -> rows [i, i+3]
    sent_quads = bass.AP(tensor=sent.ap().tensor, offset=0,
                         ap=[[D, SENT_ROWS - 3], [1, 4 * D]])

    with tile.TileContext(nc) as tc, ExitStack() as ctx:
        consts = ctx.enter_context(tc.tile_pool(name="consts", bufs=1))
        work = ctx.enter_context(tc.tile_pool(name="work", bufs=1))
        perb = ctx.enter_context(tc.tile_pool(name="perb", bufs=2))
        psums = ctx.enter_context(tc.tile_pool(name="psums", bufs=1, space="PSUM"))

        def psum_tile(name):
            return psums.tile([128, R], mybir.dt.float32, space="PSUM",
                              tag="ps", bufs=8, name=name)

        # ---- gather index table first: the Q7 is the gather's serial resource ----
        idx_t = consts.tile([128, NB, NPASS, 16], mybir.dt.int16)
        nc.sync.dma_start(out=idx_t[:], in_=gidx.ap())
        w0m_t = consts.tile([128, NQ, 3], bf)
        nc.sync.dma_start(out=w0m_t[:], in_=w0m.ap())

        # ---- quad gathers: per batch h, 2 passes of 256 quad-descriptors ----
        pair_tiles = [[None] * NPASS for _ in range(NB)]
        for h in range(NB):
            for j in range(NPASS):
                gt = work.tile([128, 2, 4 * D], bf, name=f"gp_{h}_{j}", tag=f"gp_{h}_{j}")
                nc.gpsimd.dma_gather(
                    out_ap=gt[:],
                    in_ap=sent_quads,
                    idxs_ap=idx_t[:, h, j, :],
                    num_idxs=256,
                    num_idxs_reg=256,
                    elem_size=4 * D,
                    elem_step=D,
                    single_packet=False,
                )
                pair_tiles[h][j] = gt

        # ---- constant loads (one DMA each) ----
        w1_t = consts.tile([128, NKC, HID], bf)
        nc.sync.dma_start(out=w1_t[:], in_=w1.ap())
        w2_t = consts.tile([128, HID // 128, REL], bf)
        nc.sync.dma_start(out=w2_t[:], in_=w2.ap())
        b2_t = consts.tile([REL, 1], f32)
        nc.sync.dma_start(out=b2_t[:], in_=b2t.ap())
        inv_t = consts.tile([V, NB], f32)
        nc.sync.dma_start(out=inv_t[:], in_=invcnt.ap())
        pt_t = consts.tile([128, NB, 2, V], bf)
        nc.sync.dma_start(out=pt_t[:], in_=poolt.ap())
        hs_t = consts.tile([V, NB, R], bf)
        nc.sync.dma_start(out=hs_t[:], in_=hsel.ap())
        ts_t = consts.tile([V, NB, R], bf)
        nc.sync.dma_start(out=ts_t[:], in_=tsel.ap())
        eh_t = consts.tile([128, NB, R], bf)
        nc.vector.memset(eh_t[:], 0.0)
        nc.sync.dma_start(out=eh_t[:DIS], in_=eht.ap())
        et_t = consts.tile([128, NB, R], bf)
        nc.vector.memset(et_t[:], 0.0)
        nc.sync.dma_start(out=et_t[:DIS], in_=ett.ap())

        # ---- max-tree per batch: quads -> span_emb q-slices ----
        sem_b = []  # sem_b[h][p, cc, :] = span_emb[(2h+cc)*128 + p]
        for h in range(NB):
            g0 = pair_tiles[h][0][:].rearrange("p q (r d) -> p q r d", r=4)
            g1 = pair_tiles[h][1][:].rearrange("p q (r d) -> p q r d", r=4)
            # kill quad-0 rows past each span's width (rows 1..3)
            fix = work.tile([128, 2, 3, D], bf, name=f"fix_{h}", tag=f"fix_{h}")
            mask = w0m_t[:, 2 * h : 2 * h + 2, :].broadcast_to([128, 2, 3, D])
            nc.vector.tensor_tensor(out=fix[:], in0=g0[:, :, 1:4, :], in1=mask,
                                    op=mybir.AluOpType.add)
            m1a = work.tile([128, 2, D], bf, name=f"m1a_{h}", tag=f"m1a_{h}")
            nc.vector.tensor_tensor(out=m1a[:], in0=g0[:, :, 0, :], in1=fix[:, :, 0, :], op=MAX)
            m1b = work.tile([128, 2, D], bf, name=f"m1b_{h}", tag=f"m1b_{h}")
            nc.vector.tensor_tensor(out=m1b[:], in0=fix[:, :, 1, :], in1=fix[:, :, 2, :], op=MAX)
            m2a = work.tile([128, 2, D], bf, name=f"m2a_{h}", tag=f"m2a_{h}")
            nc.vector.tensor_tensor(out=m2a[:], in0=g1[:, :, 0, :], in1=g1[:, :, 1, :], op=MAX)
            m2b = work.tile([128, 2, D], bf, name=f"m2b_{h}", tag=f"m2b_{h}")
            nc.vector.tensor_tensor(out=m2b[:], in0=g1[:, :, 2, :], in1=g1[:, :, 3, :], op=MAX)
            nc.vector.tensor_tensor(out=m1a[:], in0=m1a[:], in1=m1b[:], op=MAX)
            nc.vector.tensor_tensor(out=m2a[:], in0=m2a[:], in1=m2b[:], op=MAX)
            sh = work.tile([128, 2, D], bf, name=f"sem_{h}", tag=f"sem_{h}")
            nc.vector.tensor_tensor(out=sh[:], in0=m1a[:], in1=m2a[:], op=MAX)
            sem_b.append(sh)

        # ---- per-batch compute, batch-interleaved so the PE stays fed ----
        v_sbs, head_ts, tail_ts, prod_ts, hid_ts = {}, {}, {}, {}, {}
        for b in range(NB):
            ps_v = [psum_tile("ps_v1"), psum_tile("ps_v2")]
            for cc in range(2):
                for ni, (n0, nsz) in enumerate(((0, 512), (512, 256))):
                    nc.tensor.matmul(
                        ps_v[ni][:, :nsz],
                        lhsT=pt_t[:, b, cc, :],
                        rhs=sem_b[b][:, cc, n0 : n0 + nsz],
                        start=(cc == 0),
                        stop=(cc == 1),
                    )
            v_sb = perb.tile([V, D], bf, tag="v_sb")
            nc.scalar.activation(v_sb[:, 0:512], ps_v[0][:, :], AF.Copy, scale=inv_t[:, b : b + 1])
            nc.scalar.activation(v_sb[:, 512:768], ps_v[1][:, :256], AF.Copy, scale=inv_t[:, b : b + 1])
            v_sbs[b] = v_sb

        for b in range(NB):
            head_t = perb.tile([128, 6, R], bf, tag="head_t")
            tail_t = perb.tile([128, 6, R], bf, tag="tail_t")
            for m in range(6):
                ps_h = psum_tile("ps_h")
                nc.tensor.matmul(ps_h[:], lhsT=v_sbs[b][:, m * 128 : (m + 1) * 128],
                                 rhs=hs_t[:, b, :], start=True, stop=True)
                nc.any.tensor_copy(head_t[:, m, :], ps_h[:])
                ps_t2 = psum_tile("ps_t2")
                nc.tensor.matmul(ps_t2[:], lhsT=v_sbs[b][:, m * 128 : (m + 1) * 128],
                                 rhs=ts_t[:, b, :], start=True, stop=True)
                nc.any.tensor_copy(tail_t[:, m, :], ps_t2[:])
            prod_t = perb.tile([128, 6, R], bf, tag="prod_t")
            nc.vector.tensor_tensor(out=prod_t[:], in0=head_t[:], in1=tail_t[:],
                                    op=mybir.AluOpType.mult)
            head_ts[b], tail_ts[b], prod_ts[b] = head_t, tail_t, prod_t

        for b in range(NB):
            rhs_chunks = [head_ts[b][:, m, :] for m in range(6)]
            rhs_chunks.append(eh_t[:, b, :])
            rhs_chunks += [tail_ts[b][:, m, :] for m in range(6)]
            rhs_chunks.append(et_t[:, b, :])
            rhs_chunks += [prod_ts[b][:, m, :] for m in range(6)]

            hid_t = perb.tile([128, 3, R], bf, tag="hid_t")
            for m3 in range(3):
                ps_hid = psum_tile("ps_hid")
                for i, rhs_ap in enumerate(rhs_chunks):
                    nc.tensor.matmul(
                        ps_hid[:],
                        lhsT=w1_t[:, i, m3 * 128 : (m3 + 1) * 128],
                        rhs=rhs_ap,
                        start=(i == 0),
                        stop=(i == NKC - 1),
                    )
                nc.scalar.activation(hid_t[:, m3, :], ps_hid[:], AF.Relu)
            hid_ts[b] = hid_t

        for b in range(NB):
            ps_o = psum_tile("ps_o")
            for kc in range(3):
                nc.tensor.matmul(
                    ps_o[:REL, :], lhsT=w2_t[:, kc, :], rhs=hid_ts[b][:, kc, :],
                    start=(kc == 0), stop=(kc == 2),
                )
            out_sb = perb.tile([REL, R], f32, tag="out_sb")
            nc.scalar.activation(out_sb[:], ps_o[:REL, :], AF.Identity, bias=b2_t[:, 0:1])
            nc.sync.dma_start(out=outd.ap()[b], in_=out_sb[:])

    nc.compile()
    return nc


def _prep_core(c, sentence_repr, esi, vidx, vmask, ht, dis_h, dis_t,
               dis_embed_b, w1_p, w2_p, b2_f):
    """Build the per-core input map for batches [c*NB, c*NB+NB)."""
    bs = range(c * NB, c * NB + NB)

    sent = np.empty((SENT_ROWS, D), dtype=BF16)
    for j, b in enumerate(bs):
        sent[j * S : (j + 1) * S] = sentence_repr[b].astype(BF16)
    sent[NEGROW:] = BF16(NEG)

    # quad-gather tables: batch h, pass j, span i = q*128+p (local);
    # pass 0 base = start (rows past w masked); pass 1 base = start+min(4, w-3),
    # valid only for w>=3 (else NEG quad)
    starts = np.stack([esi[b, :, 0] for b in bs])                 # (NB, NS)
    widths = np.stack([esi[b, :, 1] - esi[b, :, 0] for b in bs])  # (NB, NS)
    gidx = np.empty((128, NB, NPASS, 16), dtype=np.int16)
    for h in range(NB):
        st, w = starts[h], widths[h]
        for j in range(NPASS):
            if j == 0:
                idx = st + h * S
            else:
                idx = np.where(w >= 3, st + np.minimum(4, w - 3) + h * S, NEGROW)
            flat = idx.astype(np.int16)                           # i = q*128+p order
            gidx[:, h, j, :] = np.tile(flat.reshape(-1, 16).T, (8, 1))

    # additive mask for quad-0 rows 1..3: -2e30 where row r > width
    w0mv = np.zeros((128, NQ, 3), dtype=BF16)
    wq = widths.reshape(NQ, 128).T                                 # [p, q]
    for r in (1, 2, 3):
        w0mv[:, :, r - 1] = np.where(wq < r, BF16(-2e30), BF16(0.0))

    poolt = np.zeros((128, NB, 2, V), dtype=BF16)
    invcnt = np.zeros((V, NB), dtype=np.float32)
    hsel = np.zeros((V, NB, R), dtype=BF16)
    tsel = np.zeros((V, NB, R), dtype=BF16)
    eht = np.empty((DIS, NB, R), dtype=BF16)
    ett = np.empty((DIS, NB, R), dtype=BF16)
    for j, b in enumerate(bs):
        pt = np.zeros((NS, V), dtype=np.float32)
        np.add.at(pt, (vidx[b].ravel(), np.repeat(np.arange(V), C)), vmask[b].ravel().astype(np.float32))
        poolt[:, j] = pt.reshape(2, 128, V).transpose(1, 0, 2).astype(BF16)
        invcnt[:, j] = 1.0 / np.maximum(vmask[b].sum(axis=1).astype(np.float32), 1.0)
        hsel[ht[b, :, 0], j, np.arange(R)] = BF16(1.0)
        tsel[ht[b, :, 1], j, np.arange(R)] = BF16(1.0)
        eht[:, j] = dis_embed_b[dis_h[b]].T
        ett[:, j] = dis_embed_b[dis_t[b]].T

    return dict(
        sent=sent, gidx=gidx, w0m=w0mv, poolt=poolt, invcnt=invcnt,
        hsel=hsel, tsel=tsel, eht=eht, ett=ett,
        w1=w1_p, w2=w2_p, b2t=b2_f,
    )


def run(trace=False, **inputs):
    global _NC_CACHE
    sentence_repr = np.asarray(inputs["sentence_repr"], dtype=np.float32)
    esi = np.asarray(inputs["entity_span_indices"]).astype(np.int64)
    vidx = np.asarray(inputs["vertex_indices"]).astype(np.int64)
    vmask = np.asarray(inputs["vertex_indices_mask"]).astype(np.int64)
    ht = np.asarray(inputs["head_tail_indices"]).astype(np.int64)
    dis_h = np.asarray(inputs["dis_h_2_t"]).astype(np.int64)
    dis_t = np.asarray(inputs["dis_t_2_h"]).astype(np.int64)
    dis_embed = np.asarray(inputs["dis_embed"], dtype=np.float32)
    w1 = np.asarray(inputs["W1"], dtype=np.float32)
    w2 = np.asarray(inputs["W2"], dtype=np.float32)
    b2 = np.asarray(inputs["b2"], dtype=np.float32)

    dis_embed_b = dis_embed.astype(BF16)
    # zero-pad W1 blocks to 20 uniform 128-row chunks, laid out [p, chunk, :]
    w1_pad = np.zeros((W1PAD, HID), dtype=BF16)
    dst = 0
    for r0, r1 in FEAT_BLOCKS:
        rows = r1 - r0
        nch = (rows + 127) // 128
        for i in range(nch):
            a = r0 + i * 128
            n = min(128, r1 - a)
            w1_pad[dst : dst + n] = w1[a : a + n].astype(BF16)
            dst += 128
    assert dst == W1PAD
    w1_p = np.ascontiguousarray(w1_pad.reshape(NKC, 128, HID).transpose(1, 0, 2))
    w2_p = np.ascontiguousarray(w2.astype(BF16).reshape(HID // 128, 128, REL).transpose(1, 0, 2))
    b2_f = b2.reshape(REL, 1).astype(np.float32)

    in_maps = [
        _prep_core(c, sentence_repr, esi, vidx, vmask, ht, dis_h, dis_t,
                   dis_embed_b, w1_p, w2_p, b2_f)
        for c in range(NCORES)
    ]

    if _NC_CACHE is None:
        _NC_CACHE = _build()

    res = bass_utils.run_bass_kernel_spmd(
        _NC_CACHE, in_maps, core_ids=list(range(NCORES)), trace=trace
    )

    out = np.empty((B, R, REL), dtype=np.float32)
    for c in range(NCORES):
        o = np.asarray(res.results[c]["outd"], dtype=np.float32)  # (NB, REL, R)
        for j in range(NB):
            out[c * NB + j] = o[j].T
    return out, res


def kernel(**inputs):
    out, _ = run(**inputs)
    return out


# revision 11
# speedup vs baseline: 1.1572x; 1.1572x over previous
"""Trainium2 Bass kernel for nn_BiLSTM_M_61615600828569 (segment_reduce).

Full computation per batch:
  span_emb = masked-max-pool of token windows   (B,256,768)
  vertex_emb = masked-mean over coref spans     (B,128,768)
  head/tail  = vertex gather by relation        (B,512,768)
  feat = [head, eh, tail, et, head*tail]        (B,512,2344)
  out  = relu(feat @ W1) @ W2 + b2              (B,512,97)

Sharding: data-parallel over batch; 16 batches / 8 cores = 2 per core.
All index work (gather tables, one-hot select matrices, pooling weights)
is precomputed on host; all float math runs on device in bf16 with fp32
PSUM accumulation, in transposed layout (features on partitions) so the
final predict.T has the 97 classes on partitions for a per-partition
bias add.

Span pooling: token rows are fetched with dma_gather at QUAD granularity
(elem = 4 overlapping rows via elem_step) — pass 0 reads rows start..start+3
(rows past the width are killed by one broadcast additive -2e30 mask on the
DVE); pass 1 reads start+min(4, w-3).. which with pass 0 covers
[start, start+w] for w>=3, and is redirected to a staged NEG quad for w<3.
Passes are split per batch so batch-0 compute starts while batch-1 still
gathers.
W1 is zero-padded to 20 uniform 128-row contraction chunks so the eh/et
blocks ride the same accumulation loop (their rhs rows past row 19 are
zeros times zero weights).
"""
import numpy as np
import ml_dtypes
from contextlib import ExitStack

import concourse.bass as bass
import concourse.bacc as bacc
import concourse.tile as tile
from concourse import mybir
from concourse import bass_utils

BF16 = ml_dtypes.bfloat16

B, S, D = 16, 1024, 768
NS, MAXW = 256, 8
V, C = 128, 6
R = 512
REL, HID, DIS = 97, 384, 20
NEG = -1e30

NCORES = 8
NB = B // NCORES          # batches per core = 2
GS = NB * NS              # spans per core = 512
NQ = GS // 128            # span groups = 4
NPASS = 2                 # quad passes per batch
SENT_ROWS = NB * S + 4    # staged sentence rows + four NEG rows (NEG quad)
NEGROW = NB * S
NKC = 20                  # uniform 128-row W1 contraction chunks
W1PAD = NKC * 128

FEAT_BLOCKS = [(0, 768), (768, 788), (788, 1556), (1556, 1576), (1576, 2344)]


def _patch_drain_and_barrier():
    """Walrus rejects >1 explicit sync wait on a Drain (TPB_CTRL), but Tile's
    tail drain waits on every used proc sem at once. Emit one single-wait
    drain per proc instead; the final drain then needs no waits."""
    import concourse.tile as tile_mod
    from concourse.vector_clock import VectorClock, ScopedClock

    if getattr(tile_mod.TileContext, "_ant_drain_patched", False):
        return

    def _patched(self, tick_clock, wait_clock):
        full = tick_clock.global_clock
        n = len(full)
        engines = [self.nc.sync, self.nc.vector, self.nc.scalar,
                   self.nc.tensor, self.nc.gpsimd]
        for i, p in enumerate([q for q in range(n) if full[q] > 0]):
            vec = [full[q] if q == p else 0 for q in range(n)]
            d = engines[i % len(engines)].drain()
            wait_clock.add_sem_waits(d.ins, ScopedClock({None: VectorClock(vec)}))
        self.nc.sync.drain()
        self.nc.all_engine_barrier()
        popped = self.nc._tile_sem_poison_stack.pop()
        assert popped is self._sem_poison
        self.nc.clear_and_free_semaphores(list(self.sems.allocated().values()))
        self.nc.all_engine_barrier()

    tile_mod.TileContext._drain_and_barrier = _patched
    tile_mod.TileContext._ant_drain_patched = True


_patch_drain_and_barrier()

_NC_CACHE = None


def _build():
    """One-core program; SPMD-replicated across the 8 cores."""
    bf = mybir.dt.bfloat16
    f32 = mybir.dt.float32
    AF = mybir.ActivationFunctionType
    MAX = mybir.AluOpType.max

    nc = bacc.Bacc("TRN2", target_bir_lowering=False, debug=False, num_devices=1)

    sent = nc.dram_tensor("sent", (SENT_ROWS, D), bf, kind="ExternalInput")
    gidx = nc.dram_tensor("gidx", (128, NB, NPASS, 16), mybir.dt.int16, kind="ExternalInput")
    w0m = nc.dram_tensor("w0m", (128, NQ, 3), bf, kind="ExternalInput")
    poolt = nc.dram_tensor("poolt", (128, NB, 2, V), bf, kind="ExternalInput")
    invcnt = nc.dram_tensor("invcnt", (V, NB), f32, kind="ExternalInput")
    hsel = nc.dram_tensor("hsel", (V, NB, R), bf, kind="ExternalInput")
    tsel = nc.dram_tensor("tsel", (V, NB, R), bf, kind="ExternalInput")
    eht = nc.dram_tensor("eht", (DIS, NB, R), bf, kind="ExternalInput")
    ett = nc.dram_tensor("ett", (DIS, NB, R), bf, kind="ExternalInput")
    w1 = nc.dram_tensor("w1", (128, NKC, HID), bf, kind="ExternalInput")
    w2 = nc.dram_tensor("w2", (128, HID // 128, REL), bf, kind="ExternalInput")
    b2t = nc.dram_tensor("b2t", (REL, 1), f32, kind="ExternalInput")
    outd = nc.dram_tensor("outd", (128, NB, R), f32, kind="ExternalOutput")

    # overlapping-quad view of the staged sentence: row i -> rows [i, i+3]
    sent_quads = bass.AP(tensor=sent.ap().tensor, offset=0,
                         ap=[[D, SENT_ROWS - 3], [1, 4 * D]])

    with tile.TileContext(nc) as tc, ExitStack() as ctx:
        consts = ctx.enter_context(tc.tile_pool(name="consts", bufs=1))
        work = ctx.enter_context(tc.tile_pool(name="work", bufs=1))
        perb = ctx.enter_context(tc.tile_pool(name="perb", bufs=2))
        psums = ctx.enter_context(tc.tile_pool(name="psums", bufs=1, space="PSUM"))

        def psum_tile(name):
            return psums.tile([128, R], mybir.dt.float32, space="PSUM",
                              tag="ps", bufs=8, name=name)

        # ---- gather index table first: the Q7 is the gather's serial resource ----
        idx_t = consts.tile([128, NB, NPASS, 16], mybir.dt.int16)
        nc.sync.dma_start(out=idx_t[:], in_=gidx.ap())
        w0m_t = consts.tile([128, NQ, 3], bf)
        nc.sync.dma_start(out=w0m_t[:], in_=w0m.ap())

        # ---- quad gathers: per batch h, 2 passes of 256 quad-descriptors ----
        pair_tiles = [[None] * NPASS for _ in range(NB)]
        for h in range(NB):
            for j in range(NPASS):
                gt = work.tile([128, 2, 4 * D], bf, name=f"gp_{h}_{j}", tag=f"gp_{h}_{j}")
                nc.gpsimd.dma_gather(
                    out_ap=gt[:],
                    in_ap=sent_quads,
                    idxs_ap=idx_t[:, h, j, :],
                    num_idxs=256,
                    num_idxs_reg=256,
                    elem_size=4 * D,
                    elem_step=D,
                    single_packet=False,
                )
                pair_tiles[h][j] = gt

        # ---- constant loads (one DMA each) ----
        w1_t = consts.tile([128, NKC, HID], bf)
        nc.sync.dma_start(out=w1_t[:], in_=w1.ap())
        w2_t = consts.tile([128, HID // 128, REL], bf)
        nc.sync.dma_start(out=w2_t[:], in_=w2.ap())
        b2_t = consts.tile([REL, 1], f32)
        nc.sync.dma_start(out=b2_t[:], in_=b2t.ap())
        inv_t = consts.tile([V, NB], f32)
        nc.sync.dma_start(out=inv_t[:], in_=invcnt.ap())
        pt_t = consts.tile([128, NB, 2, V], bf)
        nc.sync.dma_start(out=pt_t[:], in_=poolt.ap())
        hs_t = consts.tile([V, NB, R], bf)
        nc.sync.dma_start(out=hs_t[:], in_=hsel.ap())
        ts_t = consts.tile([V, NB, R], bf)
        nc.sync.dma_start(out=ts_t[:], in_=tsel.ap())
        eh_t = consts.tile([128, NB, R], bf)
        nc.vector.memset(eh_t[:], 0.0)
        nc.sync.dma_start(out=eh_t[:DIS], in_=eht.ap())
        et_t = consts.tile([128, NB, R], bf)
        nc.vector.memset(et_t[:], 0.0)
        nc.sync.dma_start(out=et_t[:DIS], in_=ett.ap())

        # ---- max-tree per batch: quads -> span_emb q-slices ----
        sem_b = []  # sem_b[h][p, cc, :] = span_emb[(2h+cc)*128 + p]
        for h in range(NB):
            g0 = pair_tiles[h][0][:].rearrange("p q (r d) -> p q r d", r=4)
            g1 = pair_tiles[h][1][:].rearrange("p q (r d) -> p q r d", r=4)
            # kill quad-0 rows past each span's width (rows 1..3)
            fix = work.tile([128, 2, 3, D], bf, name=f"fix_{h}", tag=f"fix_{h}")
            mask = w0m_t[:, 2 * h : 2 * h + 2, :].broadcast_to([128, 2, 3, D])
            nc.vector.tensor_tensor(out=fix[:], in0=g0[:, :, 1:4, :], in1=mask,
                                    op=mybir.AluOpType.add)
            m1a = work.tile([128, 2, D], bf, name=f"m1a_{h}", tag=f"m1a_{h}")
            nc.vector.tensor_tensor(out=m1a[:], in0=g0[:, :, 0, :], in1=fix[:, :, 0, :], op=MAX)
            m1b = work.tile([128, 2, D], bf, name=f"m1b_{h}", tag=f"m1b_{h}")
            nc.vector.tensor_tensor(out=m1b[:], in0=fix[:, :, 1, :], in1=fix[:, :, 2, :], op=MAX)
            m2a = work.tile([128, 2, D], bf, name=f"m2a_{h}", tag=f"m2a_{h}")
            nc.vector.tensor_tensor(out=m2a[:], in0=g1[:, :, 0, :], in1=g1[:, :, 1, :], op=MAX)
            m2b = work.tile([128, 2, D], bf, name=f"m2b_{h}", tag=f"m2b_{h}")
            nc.vector.tensor_tensor(out=m2b[:], in0=g1[:, :, 2, :], in1=g1[:, :, 3, :], op=MAX)
            nc.vector.tensor_tensor(out=m1a[:], in0=m1a[:], in1=m1b[:], op=MAX)
            nc.vector.tensor_tensor(out=m2a[:], in0=m2a[:], in1=m2b[:], op=MAX)
            sh = work.tile([128, 2, D], bf, name=f"sem_{h}", tag=f"sem_{h}")
            nc.vector.tensor_tensor(out=sh[:], in0=m1a[:], in1=m2a[:], op=MAX)
            sem_b.append(sh)

        # ---- per-batch compute, batch-interleaved so the PE stays fed ----
        v_sbs, head_ts, tail_ts, prod_ts, hid_ts = {}, {}, {}, {}, {}
        for b in range(NB):
            ps_v = [psum_tile("ps_v1"), psum_tile("ps_v2")]
            for cc in range(2):
                for ni, (n0, nsz) in enumerate(((0, 512), (512, 256))):
                    nc.tensor.matmul(
                        ps_v[ni][:, :nsz],
                        lhsT=pt_t[:, b, cc, :],
                        rhs=sem_b[b][:, cc, n0 : n0 + nsz],
                        start=(cc == 0),
                        stop=(cc == 1),
                    )
            v_sb = perb.tile([V, D], bf, tag="v_sb")
            nc.scalar.activation(v_sb[:, 0:512], ps_v[0][:, :], AF.Copy, scale=inv_t[:, b : b + 1])
            nc.scalar.activation(v_sb[:, 512:768], ps_v[1][:, :256], AF.Copy, scale=inv_t[:, b : b + 1])
            v_sbs[b] = v_sb

        for b in range(NB):
            head_t = perb.tile([128, 6, R], bf, tag="head_t")
            tail_t = perb.tile([128, 6, R], bf, tag="tail_t")
            for m in range(6):
                ps_h = psum_tile("ps_h")
                nc.tensor.matmul(ps_h[:], lhsT=v_sbs[b][:, m * 128 : (m + 1) * 128],
                                 rhs=hs_t[:, b, :], start=True, stop=True)
                nc.any.tensor_copy(head_t[:, m, :], ps_h[:])
                ps_t2 = psum_tile("ps_t2")
                nc.tensor.matmul(ps_t2[:], lhsT=v_sbs[b][:, m * 128 : (m + 1) * 128],
                                 rhs=ts_t[:, b, :], start=True, stop=True)
                nc.any.tensor_copy(tail_t[:, m, :], ps_t2[:])
            prod_t = perb.tile([128, 6, R], bf, tag="prod_t")
            nc.vector.tensor_tensor(out=prod_t[:], in0=head_t[:], in1=tail_t[:],
                                    op=mybir.AluOpType.mult)
            head_ts[b], tail_ts[b], prod_ts[b] = head_t, tail_t, prod_t

        for b in range(NB):
            rhs_chunks = [head_ts[b][:, m, :] for m in range(6)]
            rhs_chunks.append(eh_t[:, b, :])
            rhs_chunks += [tail_ts[b][:, m, :] for m in range(6)]
            rhs_chunks.append(et_t[:, b, :])
            rhs_chunks += [prod_ts[b][:, m, :] for m in range(6)]

            hid_t = perb.tile([128, 3, R], bf, tag="hid_t")
            for m3 in range(3):
                ps_hid = psum_tile("ps_hid")
                for i, rhs_ap in enumerate(rhs_chunks):
                    nc.tensor.matmul(
                        ps_hid[:],
                        lhsT=w1_t[:, i, m3 * 128 : (m3 + 1) * 128],
                        rhs=rhs_ap,
                        start=(i == 0),
                        stop=(i == NKC - 1),
                    )
                nc.scalar.activation(hid_t[:, m3, :], ps_hid[:], AF.Relu)
            hid_ts[b] = hid_t

        out_sb = work.tile([128, NB, R], f32)
        for b in range(NB):
            ps_o = psum_tile("ps_o")
            for kc in range(3):
                nc.tensor.matmul(
                    ps_o[:REL, :], lhsT=w2_t[:, kc, :], rhs=hid_ts[b][:, kc, :],
                    start=(kc == 0), stop=(kc == 2),
                )
            nc.scalar.activation(out_sb[:REL, b, :], ps_o[:REL, :], AF.Identity, bias=b2_t[:, 0:1])
        nc.sync.dma_start(out=outd.ap(), in_=out_sb[:])

    nc.compile()
    return nc


def _prep_core(c, sentence_repr, esi, vidx, vmask, ht, dis_h, dis_t,
               dis_embed_b, w1_p, w2_p, b2_f):
    """Build the per-core input map for batches [c*NB, c*NB+NB)."""
    bs = range(c * NB, c * NB + NB)

    sent = np.empty((SENT_ROWS, D), dtype=BF16)
    for j, b in enumerate(bs):
        sent[j * S : (j + 1) * S] = sentence_repr[b].astype(BF16)
    sent[NEGROW:] = BF16(NEG)

    # quad-gather tables: batch h, pass j, span i = q*128+p (local);
    # pass 0 base = start (rows past w masked); pass 1 base = start+min(4, w-3),
    # valid only for w>=3 (else NEG quad)
    starts = np.stack([esi[b, :, 0] for b in bs])                 # (NB, NS)
    widths = np.stack([esi[b, :, 1] - esi[b, :, 0] for b in bs])  # (NB, NS)
    gidx = np.empty((128, NB, NPASS, 16), dtype=np.int16)
    for h in range(NB):
        st, w = starts[h], widths[h]
        for j in range(NPASS):
            if j == 0:
                idx = st + h * S
            else:
                idx = np.where(w >= 3, st + np.minimum(4, w - 3) + h * S, NEGROW)
            flat = idx.astype(np.int16)                           # i = q*128+p order
            gidx[:, h, j, :] = np.tile(flat.reshape(-1, 16).T, (8, 1))

    # additive mask for quad-0 rows 1..3: -2e30 where row r > width
    w0mv = np.zeros((128, NQ, 3), dtype=BF16)
    wq = widths.reshape(NQ, 128).T                                 # [p, q]
    for r in (1, 2, 3):
        w0mv[:, :, r - 1] = np.where(wq < r, BF16(-2e30), BF16(0.0))

    poolt = np.zeros((128, NB, 2, V), dtype=BF16)
    invcnt = np.zeros((V, NB), dtype=np.float32)
    hsel = np.zeros((V, NB, R), dtype=BF16)
    tsel = np.zeros((V, NB, R), dtype=BF16)
    eht = np.empty((DIS, NB, R), dtype=BF16)
    ett = np.empty((DIS, NB, R), dtype=BF16)
    for j, b in enumerate(bs):
        pt = np.zeros((NS, V), dtype=np.float32)
        np.add.at(pt, (vidx[b].ravel(), np.repeat(np.arange(V), C)), vmask[b].ravel().astype(np.float32))
        poolt[:, j] = pt.reshape(2, 128, V).transpose(1, 0, 2).astype(BF16)
        invcnt[:, j] = 1.0 / np.maximum(vmask[b].sum(axis=1).astype(np.float32), 1.0)
        hsel[ht[b, :, 0], j, np.arange(R)] = BF16(1.0)
        tsel[ht[b, :, 1], j, np.arange(R)] = BF16(1.0)
        eht[:, j] = dis_embed_b[dis_h[b]].T
        ett[:, j] = dis_embed_b[dis_t[b]].T

    return dict(
        sent=sent, gidx=gidx, w0m=w0mv, poolt=poolt, invcnt=invcnt,
        hsel=hsel, tsel=tsel, eht=eht, ett=ett,
        w1=w1_p, w2=w2_p, b2t=b2_f,
    )


def run(trace=False, **inputs):
    global _NC_CACHE
    sentence_repr = np.asarray(inputs["sentence_repr"], dtype=np.float32)
    esi = np.asarray(inputs["entity_span_indices"]).astype(np.int64)
    vidx = np.asarray(inputs["vertex_indices"]).astype(np.int64)
    vmask = np.asarray(inputs["vertex_indices_mask"]).astype(np.int64)
    ht = np.asarray(inputs["head_tail_indices"]).astype(np.int64)
    dis_h = np.asarray(inputs["dis_h_2_t"]).astype(np.int64)
    dis_t = np.asarray(inputs["dis_t_2_h"]).astype(np.int64)
    dis_embed = np.asarray(inputs["dis_embed"], dtype=np.float32)
    w1 = np.asarray(inputs["W1"], dtype=np.float32)
    w2 = np.asarray(inputs["W2"], dtype=np.float32)
    b2 = np.asarray(inputs["b2"], dtype=np.float32)

    dis_embed_b = dis_embed.astype(BF16)
    # zero-pad W1 blocks to 20 uniform 128-row chunks, laid out [p, chunk, :]
    w1_pad = np.zeros((W1PAD, HID), dtype=BF16)
    dst = 0
    for r0, r1 in FEAT_BLOCKS:
        rows = r1 - r0
        nch = (rows + 127) // 128
        for i in range(nch):
            a = r0 + i * 128
            n = min(128, r1 - a)
            w1_pad[dst : dst + n] = w1[a : a + n].astype(BF16)
            dst += 128
    assert dst == W1PAD
    w1_p = np.ascontiguousarray(w1_pad.reshape(NKC, 128, HID).transpose(1, 0, 2))
    w2_p = np.ascontiguousarray(w2.astype(BF16).reshape(HID // 128, 128, REL).transpose(1, 0, 2))
    b2_f = b2.reshape(REL, 1).astype(np.float32)

    in_maps = [
        _prep_core(c, sentence_repr, esi, vidx, vmask, ht, dis_h, dis_t,
                   dis_embed_b, w1_p, w2_p, b2_f)
        for c in range(NCORES)
    ]

    if _NC_CACHE is None:
        _NC_CACHE = _build()

    res = bass_utils.run_bass_kernel_spmd(
        _NC_CACHE, in_maps, core_ids=list(range(NCORES)), trace=trace
    )

    out = np.empty((B, R, REL), dtype=np.float32)
    for c in range(NCORES):
        o = np.asarray(res.results[c]["outd"], dtype=np.float32)  # (128, NB, R)
        for j in range(NB):
            out[c * NB + j] = o[:REL, j].T
    return out, res


def kernel(**inputs):
    out, _ = run(**inputs)
    return out


# revision 13
# speedup vs baseline: 1.2832x; 1.1088x over previous
"""Trainium2 Bass kernel for nn_BiLSTM_M_61615600828569 (segment_reduce).

Full computation per batch:
  span_emb = masked-max-pool of token windows   (B,256,768)
  vertex_emb = masked-mean over coref spans     (B,128,768)
  head/tail  = vertex gather by relation        (B,512,768)
  feat = [head, eh, tail, et, head*tail]        (B,512,2344)
  out  = relu(feat @ W1) @ W2 + b2              (B,512,97)

Sharding: data-parallel over batch; 16 batches / 8 cores = 2 per core.
All index work (gather tables, one-hot select matrices, pooling weights)
is precomputed on host; all float math runs on device in bf16 with fp32
PSUM accumulation, in transposed layout (features on partitions) so the
final predict.T has the 97 classes on partitions for a per-partition
bias add.

Span pooling: token rows are fetched with dma_gather at QUAD granularity
(elem = 4 overlapping rows via elem_step) — pass 0 reads rows start..start+3
(rows past the width are killed by one broadcast additive -2e30 mask on the
DVE); pass 1 reads start+min(4, w-3).. which with pass 0 covers
[start, start+w] for w>=3, and is redirected to a staged NEG quad for w<3.
Passes are split per batch so batch-0 compute starts while batch-1 still
gathers.
W1 is zero-padded to 20 uniform 128-row contraction chunks so the eh/et
blocks ride the same accumulation loop (their rhs rows past row 19 are
zeros times zero weights).
"""
import numpy as np
import ml_dtypes
from contextlib import ExitStack

import concourse.bass as bass
import concourse.bacc as bacc
import concourse.tile as tile
from concourse import mybir
from concourse import bass_utils

BF16 = ml_dtypes.bfloat16

B, S, D = 16, 1024, 768
NS, MAXW = 256, 8
V, C = 128, 6
R = 512
REL, HID, DIS = 97, 384, 20
NEG = -1e30

NCORES = 8
NB = B // NCORES          # batches per core = 2
GS = NB * NS              # spans per core = 512
NQ = GS // 128            # span groups = 4
NPASS = 2                 # quad passes per batch
SENT_ROWS = NB * S + 4    # staged sentence rows + four NEG rows (NEG quad)
NEGROW = NB * S
NKC = 20                  # uniform 128-row W1 contraction chunks
W1PAD = NKC * 128

FEAT_BLOCKS = [(0, 768), (768, 788), (788, 1556), (1556, 1576), (1576, 2344)]


def _patch_drain_and_barrier():
    """Walrus rejects >1 explicit sync wait on a Drain (TPB_CTRL), but Tile's
    tail drain waits on every used proc sem at once. Emit one single-wait
    drain per proc instead; the final drain then needs no waits."""
    import concourse.tile as tile_mod
    from concourse.vector_clock import VectorClock, ScopedClock

    if getattr(tile_mod.TileContext, "_ant_drain_patched", False):
        return

    def _patched(self, tick_clock, wait_clock):
        full = tick_clock.global_clock
        n = len(full)
        engines = [self.nc.sync, self.nc.vector, self.nc.scalar,
                   self.nc.tensor, self.nc.gpsimd]
        for i, p in enumerate([q for q in range(n) if full[q] > 0]):
            vec = [full[q] if q == p else 0 for q in range(n)]
            d = engines[i % len(engines)].drain()
            wait_clock.add_sem_waits(d.ins, ScopedClock({None: VectorClock(vec)}))
        self.nc.sync.drain()
        self.nc.all_engine_barrier()
        popped = self.nc._tile_sem_poison_stack.pop()
        assert popped is self._sem_poison
        self.nc.clear_and_free_semaphores(list(self.sems.allocated().values()))
        self.nc.all_engine_barrier()

    tile_mod.TileContext._drain_and_barrier = _patched
    tile_mod.TileContext._ant_drain_patched = True


_patch_drain_and_barrier()

_NC_CACHE = None


def _build():
    """One-core program; SPMD-replicated across the 8 cores."""
    bf = mybir.dt.bfloat16
    f32 = mybir.dt.float32
    AF = mybir.ActivationFunctionType
    MAX = mybir.AluOpType.max

    nc = bacc.Bacc("TRN2", target_bir_lowering=False, debug=False, num_devices=1)

    sent = nc.dram_tensor("sent", (SENT_ROWS, D), bf, kind="ExternalInput")
    gidx = nc.dram_tensor("gidx", (128, NB, NPASS, 16), mybir.dt.int16, kind="ExternalInput")
    w0m = nc.dram_tensor("w0m", (128, NQ, 3), f32, kind="ExternalInput")
    poolt = nc.dram_tensor("poolt", (128, NB, 2, V), bf, kind="ExternalInput")
    invcnt = nc.dram_tensor("invcnt", (V, NB), f32, kind="ExternalInput")
    hsel = nc.dram_tensor("hsel", (V, NB, R), bf, kind="ExternalInput")
    tsel = nc.dram_tensor("tsel", (V, NB, R), bf, kind="ExternalInput")
    eht = nc.dram_tensor("eht", (DIS, NB, R), bf, kind="ExternalInput")
    ett = nc.dram_tensor("ett", (DIS, NB, R), bf, kind="ExternalInput")
    w1 = nc.dram_tensor("w1", (128, NKC, HID), bf, kind="ExternalInput")
    w2 = nc.dram_tensor("w2", (128, HID // 128, REL), bf, kind="ExternalInput")
    b2t = nc.dram_tensor("b2t", (REL, 1), f32, kind="ExternalInput")
    outd = nc.dram_tensor("outd", (128, NB, R), f32, kind="ExternalOutput")

    # overlapping-quad view of the staged sentence: row i -> rows [i, i+3]
    sent_quads = bass.AP(tensor=sent.ap().tensor, offset=0,
                         ap=[[D, SENT_ROWS - 3], [1, 4 * D]])

    with tile.TileContext(nc) as tc, ExitStack() as ctx:
        consts = ctx.enter_context(tc.tile_pool(name="consts", bufs=1))
        work = ctx.enter_context(tc.tile_pool(name="work", bufs=1))
        perb = ctx.enter_context(tc.tile_pool(name="perb", bufs=2))
        psums = ctx.enter_context(tc.tile_pool(name="psums", bufs=1, space="PSUM"))

        def psum_tile(name, tag, bufs):
            return psums.tile([128, R], mybir.dt.float32, space="PSUM",
                              tag=tag, bufs=bufs, name=name)

        # ---- gather index table first: the Q7 is the gather's serial resource ----
        idx_t = consts.tile([128, NB, NPASS, 16], mybir.dt.int16)
        nc.sync.dma_start(out=idx_t[:], in_=gidx.ap())
        w0m_t = consts.tile([128, NQ, 3], f32)
        nc.sync.dma_start(out=w0m_t[:], in_=w0m.ap())

        # ---- quad gathers: per batch h, 2 passes of 256 quad-descriptors ----
        pair_tiles = [[None] * NPASS for _ in range(NB)]
        for h in range(NB):
            for j in range(NPASS):
                gt = work.tile([128, 2, 4 * D], bf, name=f"gp_{h}_{j}", tag=f"gp_{h}_{j}")
                nc.gpsimd.dma_gather(
                    out_ap=gt[:],
                    in_ap=sent_quads,
                    idxs_ap=idx_t[:, h, j, :],
                    num_idxs=256,
                    num_idxs_reg=256,
                    elem_size=4 * D,
                    elem_step=D,
                    single_packet=False,
                )
                pair_tiles[h][j] = gt

        # ---- constant loads (one DMA each) ----
        w1_t = consts.tile([128, NKC, HID], bf)
        nc.sync.dma_start(out=w1_t[:], in_=w1.ap())
        w2_t = consts.tile([128, HID // 128, REL], bf)
        nc.sync.dma_start(out=w2_t[:], in_=w2.ap())
        b2_t = consts.tile([REL, 1], f32)
        nc.sync.dma_start(out=b2_t[:], in_=b2t.ap())
        inv_t = consts.tile([V, NB], f32)
        nc.sync.dma_start(out=inv_t[:], in_=invcnt.ap())
        pt_t = consts.tile([128, NB, 2, V], bf)
        nc.sync.dma_start(out=pt_t[:], in_=poolt.ap())
        hs_t = consts.tile([V, NB, R], bf)
        nc.sync.dma_start(out=hs_t[:], in_=hsel.ap())
        ts_t = consts.tile([V, NB, R], bf)
        nc.sync.dma_start(out=ts_t[:], in_=tsel.ap())
        eh_t = consts.tile([128, NB, R], bf)
        nc.vector.memset(eh_t[:], 0.0)
        nc.sync.dma_start(out=eh_t[:DIS], in_=eht.ap())
        et_t = consts.tile([128, NB, R], bf)
        nc.vector.memset(et_t[:], 0.0)
        nc.sync.dma_start(out=et_t[:DIS], in_=ett.ap())

        # ---- max-tree per batch: quads -> span_emb q-slices ----
        # quad-0 rows 1..3 are folded in with scalar_tensor_tensor:
        # acc = max(acc, row_r + mask_r), mask_r = -2e30 where r > width
        sem_b = []  # sem_b[h][p, cc, :] = span_emb[(2h+cc)*128 + p]
        for h in range(NB):
            g0 = pair_tiles[h][0][:].rearrange("p q (r d) -> p q r d", r=4)
            g1 = pair_tiles[h][1][:].rearrange("p q (r d) -> p q r d", r=4)
            q0m = work.tile([128, 2, D], bf, name=f"q0m_{h}", tag=f"q0m_{h}")
            for q in range(2):
                gq = 2 * h + q
                nc.vector.scalar_tensor_tensor(
                    out=q0m[:, q, :], in0=g0[:, q, 1, :], scalar=w0m_t[:, gq, 0:1],
                    in1=g0[:, q, 0, :], op0=mybir.AluOpType.add, op1=MAX)
                nc.vector.scalar_tensor_tensor(
                    out=q0m[:, q, :], in0=g0[:, q, 2, :], scalar=w0m_t[:, gq, 1:2],
                    in1=q0m[:, q, :], op0=mybir.AluOpType.add, op1=MAX)
                nc.vector.scalar_tensor_tensor(
                    out=q0m[:, q, :], in0=g0[:, q, 3, :], scalar=w0m_t[:, gq, 2:3],
                    in1=q0m[:, q, :], op0=mybir.AluOpType.add, op1=MAX)
            m2a = work.tile([128, 2, D], bf, name=f"m2a_{h}", tag=f"m2a_{h}")
            nc.vector.tensor_tensor(out=m2a[:], in0=g1[:, :, 0, :], in1=g1[:, :, 1, :], op=MAX)
            m2b = work.tile([128, 2, D], bf, name=f"m2b_{h}", tag=f"m2b_{h}")
            nc.vector.tensor_tensor(out=m2b[:], in0=g1[:, :, 2, :], in1=g1[:, :, 3, :], op=MAX)
            nc.vector.tensor_tensor(out=m2a[:], in0=m2a[:], in1=m2b[:], op=MAX)
            sh = work.tile([128, 2, D], bf, name=f"sem_{h}", tag=f"sem_{h}")
            nc.vector.tensor_tensor(out=sh[:], in0=q0m[:], in1=m2a[:], op=MAX)
            sem_b.append(sh)

        # ---- per-batch compute, batch-interleaved so the PE stays fed ----
        v_sbs, head_ts, tail_ts, prod_ts, hid_ts = {}, {}, {}, {}, {}
        for b in range(NB):
            ps_v = psums.tile([128, D], mybir.dt.float32, space="PSUM",
                              tag="ps_v", bufs=1, name="ps_v")
            for cc in range(2):
                for n0, nsz in ((0, 512), (512, 256)):
                    nc.tensor.matmul(
                        ps_v[:, n0 : n0 + nsz],
                        lhsT=pt_t[:, b, cc, :],
                        rhs=sem_b[b][:, cc, n0 : n0 + nsz],
                        start=(cc == 0),
                        stop=(cc == 1),
                    )
            v_sb = perb.tile([V, D], bf, tag="v_sb")
            nc.scalar.activation(v_sb[:], ps_v[:], AF.Copy, scale=inv_t[:, b : b + 1])
            v_sbs[b] = v_sb

        for b in range(NB):
            head_t = perb.tile([128, 6, R], bf, tag="head_t")
            tail_t = perb.tile([128, 6, R], bf, tag="tail_t")
            prod_t = perb.tile([128, 6, R], bf, tag="prod_t")
            for m in range(6):
                ps_h = psum_tile("ps_h", "sel", 3)
                nc.tensor.matmul(ps_h[:], lhsT=v_sbs[b][:, m * 128 : (m + 1) * 128],
                                 rhs=hs_t[:, b, :], start=True, stop=True)
                nc.any.tensor_copy(head_t[:, m, :], ps_h[:])
                ps_t2 = psum_tile("ps_t2", "sel", 3)
                nc.tensor.matmul(ps_t2[:], lhsT=v_sbs[b][:, m * 128 : (m + 1) * 128],
                                 rhs=ts_t[:, b, :], start=True, stop=True)
                nc.any.tensor_copy(tail_t[:, m, :], ps_t2[:])
                nc.vector.tensor_tensor(out=prod_t[:, m, :], in0=head_t[:, m, :],
                                        in1=tail_t[:, m, :], op=mybir.AluOpType.mult)
            head_ts[b], tail_ts[b], prod_ts[b] = head_t, tail_t, prod_t

        for b in range(NB):
            rhs_chunks = [head_ts[b][:, m, :] for m in range(6)]
            rhs_chunks.append(eh_t[:, b, :])
            rhs_chunks += [tail_ts[b][:, m, :] for m in range(6)]
            rhs_chunks.append(et_t[:, b, :])
            rhs_chunks += [prod_ts[b][:, m, :] for m in range(6)]

            hid_t = perb.tile([128, 3, R], bf, tag="hid_t")
            for m3 in range(3):
                ps_hid = psum_tile("ps_hid", "hid", 2)
                for i, rhs_ap in enumerate(rhs_chunks):
                    nc.tensor.matmul(
                        ps_hid[:],
                        lhsT=w1_t[:, i, m3 * 128 : (m3 + 1) * 128],
                        rhs=rhs_ap,
                        start=(i == 0),
                        stop=(i == NKC - 1),
                    )
                nc.scalar.activation(hid_t[:, m3, :], ps_hid[:], AF.Relu)
            hid_ts[b] = hid_t

        out_sb = work.tile([128, NB, R], f32)
        for b in range(NB):
            ps_o = psum_tile("ps_o", "out", 1)
            for kc in range(3):
                nc.tensor.matmul(
                    ps_o[:REL, :], lhsT=w2_t[:, kc, :], rhs=hid_ts[b][:, kc, :],
                    start=(kc == 0), stop=(kc == 2),
                )
            nc.scalar.activation(out_sb[:REL, b, :], ps_o[:REL, :], AF.Identity, bias=b2_t[:, 0:1])
        nc.sync.dma_start(out=outd.ap(), in_=out_sb[:])

    nc.compile()
    return nc


def _prep_core(c, sentence_repr, esi, vidx, vmask, ht, dis_h, dis_t,
               dis_embed_b, w1_p, w2_p, b2_f):
    """Build the per-core input map for batches [c*NB, c*NB+NB)."""
    bs = range(c * NB, c * NB + NB)

    sent = np.empty((SENT_ROWS, D), dtype=BF16)
    for j, b in enumerate(bs):
        sent[j * S : (j + 1) * S] = sentence_repr[b].astype(BF16)
    sent[NEGROW:] = BF16(NEG)

    # quad-gather tables: batch h, pass j, span i = q*128+p (local);
    # pass 0 base = start (rows past w masked); pass 1 base = start+min(4, w-3),
    # valid only for w>=3 (else NEG quad)
    starts = np.stack([esi[b, :, 0] for b in bs])                 # (NB, NS)
    widths = np.stack([esi[b, :, 1] - esi[b, :, 0] for b in bs])  # (NB, NS)
    gidx = np.empty((128, NB, NPASS, 16), dtype=np.int16)
    for h in range(NB):
        st, w = starts[h], widths[h]
        for j in range(NPASS):
            if j == 0:
                idx = st + h * S
            else:
                idx = np.where(w >= 3, st + np.minimum(4, w - 3) + h * S, NEGROW)
            flat = idx.astype(np.int16)                           # i = q*128+p order
            gidx[:, h, j, :] = np.tile(flat.reshape(-1, 16).T, (8, 1))

    # additive mask for quad-0 rows 1..3: -2e30 where row r > width
    w0mv = np.zeros((128, NQ, 3), dtype=np.float32)
    wq = widths.reshape(NQ, 128).T                                 # [p, q]
    for r in (1, 2, 3):
        w0mv[:, :, r - 1] = np.where(wq < r, np.float32(-2e30), np.float32(0.0))

    poolt = np.zeros((128, NB, 2, V), dtype=BF16)
    invcnt = np.zeros((V, NB), dtype=np.float32)
    hsel = np.zeros((V, NB, R), dtype=BF16)
    tsel = np.zeros((V, NB, R), dtype=BF16)
    eht = np.empty((DIS, NB, R), dtype=BF16)
    ett = np.empty((DIS, NB, R), dtype=BF16)
    for j, b in enumerate(bs):
        pt = np.zeros((NS, V), dtype=np.float32)
        np.add.at(pt, (vidx[b].ravel(), np.repeat(np.arange(V), C)), vmask[b].ravel().astype(np.float32))
        poolt[:, j] = pt.reshape(2, 128, V).transpose(1, 0, 2).astype(BF16)
        invcnt[:, j] = 1.0 / np.maximum(vmask[b].sum(axis=1).astype(np.float32), 1.0)
        hsel[ht[b, :, 0], j, np.arange(R)] = BF16(1.0)
        tsel[ht[b, :, 1], j, np.arange(R)] = BF16(1.0)
        eht[:, j] = dis_embed_b[dis_h[b]].T
        ett[:, j] = dis_embed_b[dis_t[b]].T

    return dict(
        sent=sent, gidx=gidx, w0m=w0mv, poolt=poolt, invcnt=invcnt,
        hsel=hsel, tsel=tsel, eht=eht, ett=ett,
        w1=w1_p, w2=w2_p, b2t=b2_f,
    )


def run(trace=False, **inputs):
    global _NC_CACHE
    sentence_repr = np.asarray(inputs["sentence_repr"], dtype=np.float32)
    esi = np.asarray(inputs["entity_span_indices"]).astype(np.int64)
    vidx = np.asarray(inputs["vertex_indices"]).astype(np.int64)
    vmask = np.asarray(inputs["vertex_indices_mask"]).astype(np.int64)
    ht = np.asarray(inputs["head_tail_indices"]).astype(np.int64)
    dis_h = np.asarray(inputs["dis_h_2_t"]).astype(np.int64)
    dis_t = np.asarray(inputs["dis_t_2_h"]).astype(np.int64)
    dis_embed = np.asarray(inputs["dis_embed"], dtype=np.float32)
    w1 = np.asarray(inputs["W1"], dtype=np.float32)
    w2 = np.asarray(inputs["W2"], dtype=np.float32)
    b2 = np.asarray(inputs["b2"], dtype=np.float32)

    dis_embed_b = dis_embed.astype(BF16)
    # zero-pad W1 blocks to 20 uniform 128-row chunks, laid out [p, chunk, :]
    w1_pad = np.zeros((W1PAD, HID), dtype=BF16)
    dst = 0
    for r0, r1 in FEAT_BLOCKS:
        rows = r1 - r0
        nch = (rows + 127) // 128
        for i in range(nch):
            a = r0 + i * 128
            n = min(128, r1 - a)
            w1_pad[dst : dst + n] = w1[a : a + n].astype(BF16)
            dst += 128
    assert dst == W1PAD
    w1_p = np.ascontiguousarray(w1_pad.reshape(NKC, 128, HID).transpose(1, 0, 2))
    w2_p = np.ascontiguousarray(w2.astype(BF16).reshape(HID // 128, 128, REL).transpose(1, 0, 2))
    b2_f = b2.reshape(REL, 1).astype(np.float32)

    in_maps = [
        _prep_core(c, sentence_repr, esi, vidx, vmask, ht, dis_h, dis_t,
                   dis_embed_b, w1_p, w2_p, b2_f)
        for c in range(NCORES)
    ]

    if _NC_CACHE is None:
        _NC_CACHE = _build()

    res = bass_utils.run_bass_kernel_spmd(
        _NC_CACHE, in_maps, core_ids=list(range(NCORES)), trace=trace
    )

    out = np.empty((B, R, REL), dtype=np.float32)
    for c in range(NCORES):
        o = np.asarray(res.results[c]["outd"], dtype=np.float32)  # (128, NB, R)
        for j in range(NB):
            out[c * NB + j] = o[:REL, j].T
    return out, res


def kernel(**inputs):
    out, _ = run(**inputs)
    return out


# revision 15
# speedup vs baseline: 1.3179x; 1.0271x over previous
"""Trainium2 Bass kernel for nn_BiLSTM_M_61615600828569 (segment_reduce).

Full computation per batch:
  span_emb = masked-max-pool of token windows   (B,256,768)
  vertex_emb = masked-mean over coref spans     (B,128,768)
  head/tail  = vertex gather by relation        (B,512,768)
  feat = [head, eh, tail, et, head*tail]        (B,512,2344)
  out  = relu(feat @ W1) @ W2 + b2              (B,512,97)

Sharding: data-parallel over batch; 16 batches / 8 cores = 2 per core.
All index work (gather tables, one-hot select matrices, pooling weights)
is precomputed on host; all float math runs on device in bf16 with fp32
PSUM accumulation, in transposed layout (features on partitions) so the
final predict.T has the 97 classes on partitions for a per-partition
bias add.

Span pooling: token rows are fetched with dma_gather at QUAD granularity
(elem = 4 overlapping rows via elem_step) — pass 0 reads rows start..start+3
(rows past the width are killed by one broadcast additive -2e30 mask on the
DVE); pass 1 reads start+min(4, w-3).. which with pass 0 covers
[start, start+w] for w>=3, and is redirected to a staged NEG quad for w<3.
Passes are split per batch so batch-0 compute starts while batch-1 still
gathers.
W1 is zero-padded to 20 uniform 128-row contraction chunks so the eh/et
blocks ride the same accumulation loop (their rhs rows past row 19 are
zeros times zero weights).
"""
import numpy as np
import ml_dtypes
from contextlib import ExitStack

import concourse.bass as bass
import concourse.bacc as bacc
import concourse.tile as tile
from concourse import mybir
from concourse import bass_utils

BF16 = ml_dtypes.bfloat16

B, S, D = 16, 1024, 768
NS, MAXW = 256, 8
V, C = 128, 6
R = 512
REL, HID, DIS = 97, 384, 20
NEG = -1e30

NCORES = 8
NB = B // NCORES          # batches per core = 2
GS = NB * NS              # spans per core = 512
NQ = GS // 128            # span groups = 4
NPASS = 2                 # quad passes per batch
SENT_ROWS = NB * S + 4    # staged sentence rows + four NEG rows (NEG quad)
NEGROW = NB * S
NKC = 20                  # uniform 128-row W1 contraction chunks
W1PAD = NKC * 128

FEAT_BLOCKS = [(0, 768), (768, 788), (788, 1556), (1556, 1576), (1576, 2344)]


def _patch_drain_and_barrier():
    """Walrus rejects >1 explicit sync wait on a Drain (TPB_CTRL), but Tile's
    tail drain waits on every used proc sem at once. Emit one single-wait
    drain per proc instead; the final drain then needs no waits."""
    import concourse.tile as tile_mod
    from concourse.vector_clock import VectorClock, ScopedClock

    if getattr(tile_mod.TileContext, "_ant_drain_patched", False):
        return

    def _patched(self, tick_clock, wait_clock):
        full = tick_clock.global_clock
        n = len(full)
        engines = [self.nc.sync, self.nc.vector, self.nc.scalar,
                   self.nc.tensor, self.nc.gpsimd]
        for i, p in enumerate([q for q in range(n) if full[q] > 0]):
            vec = [full[q] if q == p else 0 for q in range(n)]
            d = engines[i % len(engines)].drain()
            wait_clock.add_sem_waits(d.ins, ScopedClock({None: VectorClock(vec)}))
        self.nc.sync.drain()
        self.nc.all_engine_barrier()
        popped = self.nc._tile_sem_poison_stack.pop()
        assert popped is self._sem_poison
        self.nc.clear_and_free_semaphores(list(self.sems.allocated().values()))
        self.nc.all_engine_barrier()

    tile_mod.TileContext._drain_and_barrier = _patched
    tile_mod.TileContext._ant_drain_patched = True


_patch_drain_and_barrier()

_NC_CACHE = None


def _build():
    """One-core program; SPMD-replicated across the 8 cores."""
    bf = mybir.dt.bfloat16
    f32 = mybir.dt.float32
    AF = mybir.ActivationFunctionType
    MAX = mybir.AluOpType.max

    nc = bacc.Bacc("TRN2", target_bir_lowering=False, debug=False, num_devices=1)

    sent = nc.dram_tensor("sent", (SENT_ROWS, D), bf, kind="ExternalInput")
    gidx = nc.dram_tensor("gidx", (128, NB, NPASS, 16), mybir.dt.int16, kind="ExternalInput")
    w0m = nc.dram_tensor("w0m", (128, NQ, 3), f32, kind="ExternalInput")
    poolt = nc.dram_tensor("poolt", (128, NB, 2, V), bf, kind="ExternalInput")
    invcnt = nc.dram_tensor("invcnt", (V, NB), f32, kind="ExternalInput")
    hsel = nc.dram_tensor("hsel", (V, NB, R), bf, kind="ExternalInput")
    tsel = nc.dram_tensor("tsel", (V, NB, R), bf, kind="ExternalInput")
    dist = nc.dram_tensor("dist", (DIS, DIS), bf, kind="ExternalInput")
    ehsel = nc.dram_tensor("ehsel", (DIS, NB, R), bf, kind="ExternalInput")
    etsel = nc.dram_tensor("etsel", (DIS, NB, R), bf, kind="ExternalInput")
    w1 = nc.dram_tensor("w1", (128, NKC, HID), bf, kind="ExternalInput")
    w2 = nc.dram_tensor("w2", (128, HID // 128, REL), bf, kind="ExternalInput")
    b2t = nc.dram_tensor("b2t", (REL, 1), f32, kind="ExternalInput")
    outd = nc.dram_tensor("outd", (128, NB, R), f32, kind="ExternalOutput")

    # overlapping-quad view of the staged sentence: row i -> rows [i, i+3]
    sent_quads = bass.AP(tensor=sent.ap().tensor, offset=0,
                         ap=[[D, SENT_ROWS - 3], [1, 4 * D]])

    with tile.TileContext(nc) as tc, ExitStack() as ctx:
        consts = ctx.enter_context(tc.tile_pool(name="consts", bufs=1))
        work = ctx.enter_context(tc.tile_pool(name="work", bufs=1))
        perb = ctx.enter_context(tc.tile_pool(name="perb", bufs=2))
        psums = ctx.enter_context(tc.tile_pool(name="psums", bufs=1, space="PSUM"))

        def psum_tile(name, tag, bufs):
            return psums.tile([128, R], mybir.dt.float32, space="PSUM",
                              tag=tag, bufs=bufs, name=name)

        # ---- gather index table first: the Q7 is the gather's serial resource ----
        idx_t = consts.tile([128, NB, NPASS, 16], mybir.dt.int16)
        nc.sync.dma_start(out=idx_t[:], in_=gidx.ap())
        w0m_t = consts.tile([128, NQ, 3], f32)
        nc.sync.dma_start(out=w0m_t[:], in_=w0m.ap())

        # ---- quad gathers: per batch h, 2 passes of 256 quad-descriptors ----
        pair_tiles = [[None] * NPASS for _ in range(NB)]
        for h in range(NB):
            for j in range(NPASS):
                gt = work.tile([128, 2, 4 * D], bf, name=f"gp_{h}_{j}", tag=f"gp_{h}_{j}")
                nc.gpsimd.dma_gather(
                    out_ap=gt[:],
                    in_ap=sent_quads,
                    idxs_ap=idx_t[:, h, j, :],
                    num_idxs=256,
                    num_idxs_reg=256,
                    elem_size=4 * D,
                    elem_step=D,
                    single_packet=False,
                )
                pair_tiles[h][j] = gt

        # ---- constant loads (one DMA each) ----
        w1_t = consts.tile([128, NKC, HID], bf)
        nc.sync.dma_start(out=w1_t[:], in_=w1.ap())
        w2_t = consts.tile([128, HID // 128, REL], bf)
        nc.sync.dma_start(out=w2_t[:], in_=w2.ap())
        b2_t = consts.tile([REL, 1], f32)
        nc.sync.dma_start(out=b2_t[:], in_=b2t.ap())
        inv_t = consts.tile([V, NB], f32)
        nc.sync.dma_start(out=inv_t[:], in_=invcnt.ap())
        pt_t = consts.tile([128, NB, 2, V], bf)
        nc.sync.dma_start(out=pt_t[:], in_=poolt.ap())
        hs_t = consts.tile([V, NB, R], bf)
        nc.sync.dma_start(out=hs_t[:], in_=hsel.ap())
        ts_t = consts.tile([V, NB, R], bf)
        nc.sync.dma_start(out=ts_t[:], in_=tsel.ap())
        dist_t = consts.tile([DIS, DIS], bf)
        nc.sync.dma_start(out=dist_t[:], in_=dist.ap())
        ehs_t = consts.tile([DIS, NB, R], bf)
        nc.sync.dma_start(out=ehs_t[:], in_=ehsel.ap())
        ets_t = consts.tile([DIS, NB, R], bf)
        nc.sync.dma_start(out=ets_t[:], in_=etsel.ap())

        # ---- max-tree per batch: quads -> span_emb q-slices ----
        # quad-0 rows 1..3 are folded in with scalar_tensor_tensor:
        # acc = max(acc, row_r + mask_r), mask_r = -2e30 where r > width
        sem_b = []  # sem_b[h][p, cc, :] = span_emb[(2h+cc)*128 + p]
        for h in range(NB):
            g0 = pair_tiles[h][0][:].rearrange("p q (r d) -> p q r d", r=4)
            g1 = pair_tiles[h][1][:].rearrange("p q (r d) -> p q r d", r=4)
            q0m = work.tile([128, 2, D], bf, name=f"q0m_{h}", tag=f"q0m_{h}")
            for q in range(2):
                gq = 2 * h + q
                nc.vector.scalar_tensor_tensor(
                    out=q0m[:, q, :], in0=g0[:, q, 1, :], scalar=w0m_t[:, gq, 0:1],
                    in1=g0[:, q, 0, :], op0=mybir.AluOpType.add, op1=MAX)
                nc.vector.scalar_tensor_tensor(
                    out=q0m[:, q, :], in0=g0[:, q, 2, :], scalar=w0m_t[:, gq, 1:2],
                    in1=q0m[:, q, :], op0=mybir.AluOpType.add, op1=MAX)
                nc.vector.scalar_tensor_tensor(
                    out=q0m[:, q, :], in0=g0[:, q, 3, :], scalar=w0m_t[:, gq, 2:3],
                    in1=q0m[:, q, :], op0=mybir.AluOpType.add, op1=MAX)
            m2a = work.tile([128, 2, D], bf, name=f"m2a_{h}", tag=f"m2a_{h}")
            nc.vector.tensor_tensor(out=m2a[:], in0=g1[:, :, 0, :], in1=g1[:, :, 1, :], op=MAX)
            m2b = work.tile([128, 2, D], bf, name=f"m2b_{h}", tag=f"m2b_{h}")
            nc.vector.tensor_tensor(out=m2b[:], in0=g1[:, :, 2, :], in1=g1[:, :, 3, :], op=MAX)
            nc.vector.tensor_tensor(out=m2a[:], in0=m2a[:], in1=m2b[:], op=MAX)
            sh = work.tile([128, 2, D], bf, name=f"sem_{h}", tag=f"sem_{h}")
            nc.vector.tensor_tensor(out=sh[:], in0=q0m[:], in1=m2a[:], op=MAX)
            sem_b.append(sh)

        # ---- Ew = dis_embed @ W1-block, shared by both batches ----
        ew_sbs = {}
        for name, ci in (("ewb", 6), ("ewd", 13)):
            ps_e = psums.tile([DIS, HID], mybir.dt.float32, space="PSUM",
                              tag="out", bufs=1, name=f"ps_{name}")
            nc.tensor.matmul(ps_e[:], lhsT=dist_t[:], rhs=w1_t[:DIS, ci, :],
                             start=True, stop=True)
            ew_sb = consts.tile([DIS, HID], bf, name=f"{name}_sb")
            nc.scalar.activation(ew_sb[:], ps_e[:], AF.Copy)
            ew_sbs[name] = ew_sb

        # ---- per-batch compute, batch-interleaved so the PE stays fed ----
        v_sbs, vw_sbs, head_ts, tail_ts, prod_ts, hid_ts = {}, {}, {}, {}, {}, {}
        for b in range(NB):
            ps_v = psums.tile([128, D], mybir.dt.float32, space="PSUM",
                              tag="ps_v", bufs=1, name="ps_v")
            for cc in range(2):
                for n0, nsz in ((0, 512), (512, 256)):
                    nc.tensor.matmul(
                        ps_v[:, n0 : n0 + nsz],
                        lhsT=pt_t[:, b, cc, :],
                        rhs=sem_b[b][:, cc, n0 : n0 + nsz],
                        start=(cc == 0),
                        stop=(cc == 1),
                    )
            v_sb = perb.tile([V, D], bf, tag="v_sb")
            nc.scalar.activation(v_sb[:], ps_v[:], AF.Copy, scale=inv_t[:, b : b + 1])
            v_sbs[b] = v_sb

            # V_emb.T chunks (for Vw), then Vw_a/Vw_c = (V_emb @ W1a/c) * inv
            vt_sb = perb.tile([128, 6, V], bf, tag="vt_sb")
            for m in range(6):
                ps_vt = psum_tile(f"ps_vt", "sel", 3)
                for cc in range(2):
                    nc.tensor.matmul(ps_vt[:, :V],
                                     lhsT=sem_b[b][:, cc, m * 128 : (m + 1) * 128],
                                     rhs=pt_t[:, b, cc, :],
                                     start=(cc == 0), stop=(cc == 1))
                nc.any.tensor_copy(vt_sb[:, m, :], ps_vt[:, :V])
            vw_a = perb.tile([V, HID], bf, tag="vw_a")
            vw_c = perb.tile([V, HID], bf, tag="vw_c")
            for name, vw, c0 in (("a", vw_a, 0), ("c", vw_c, 7)):
                ps_vw = psum_tile(f"ps_vw", "sel", 3)
                for m in range(6):
                    nc.tensor.matmul(ps_vw[:, :HID], lhsT=vt_sb[:, m, :],
                                     rhs=w1_t[:, c0 + m, :],
                                     start=(m == 0), stop=(m == 5))
                nc.scalar.activation(vw[:], ps_vw[:, :HID], AF.Copy,
                                     scale=inv_t[:, b : b + 1])
            vw_sbs[b] = (vw_a, vw_c)

        for b in range(NB):
            head_t = perb.tile([128, 6, R], bf, tag="head_t")
            tail_t = perb.tile([128, 6, R], bf, tag="tail_t")
            prod_t = perb.tile([128, 6, R], bf, tag="prod_t")
            for m in range(6):
                ps_h = psum_tile("ps_h", "sel", 3)
                nc.tensor.matmul(ps_h[:], lhsT=v_sbs[b][:, m * 128 : (m + 1) * 128],
                                 rhs=hs_t[:, b, :], start=True, stop=True)
                nc.any.tensor_copy(head_t[:, m, :], ps_h[:])
                ps_t2 = psum_tile("ps_t2", "sel", 3)
                nc.tensor.matmul(ps_t2[:], lhsT=v_sbs[b][:, m * 128 : (m + 1) * 128],
                                 rhs=ts_t[:, b, :], start=True, stop=True)
                nc.any.tensor_copy(tail_t[:, m, :], ps_t2[:])
                nc.vector.tensor_tensor(out=prod_t[:, m, :], in0=head_t[:, m, :],
                                        in1=tail_t[:, m, :], op=mybir.AluOpType.mult)
            head_ts[b], tail_ts[b], prod_ts[b] = head_t, tail_t, prod_t

        for b in range(NB):
            vw_a, vw_c = vw_sbs[b]
            hid_t = perb.tile([128, 3, R], bf, tag="hid_t")
            for m3 in range(3):
                msl = slice(m3 * 128, (m3 + 1) * 128)
                chunks = [(vw_a[:, msl], hs_t[:, b, :]),
                          (ew_sbs["ewb"][:, msl], ehs_t[:, b, :]),
                          (vw_c[:, msl], ts_t[:, b, :]),
                          (ew_sbs["ewd"][:, msl], ets_t[:, b, :])]
                chunks += [(w1_t[:, 14 + m, msl], prod_ts[b][:, m, :]) for m in range(6)]
                ps_hid = psum_tile("ps_hid", "hid", 2)
                for i, (lhsT, rhs_ap) in enumerate(chunks):
                    nc.tensor.matmul(ps_hid[:], lhsT=lhsT, rhs=rhs_ap,
                                     start=(i == 0), stop=(i == len(chunks) - 1))
                nc.scalar.activation(hid_t[:, m3, :], ps_hid[:], AF.Relu)
            hid_ts[b] = hid_t

        out_sb = work.tile([128, NB, R], f32)
        for b in range(NB):
            ps_o = psum_tile("ps_o", "out", 1)
            for kc in range(3):
                nc.tensor.matmul(
                    ps_o[:REL, :], lhsT=w2_t[:, kc, :], rhs=hid_ts[b][:, kc, :],
                    start=(kc == 0), stop=(kc == 2),
                )
            nc.scalar.activation(out_sb[:REL, b, :], ps_o[:REL, :], AF.Identity, bias=b2_t[:, 0:1])
        nc.sync.dma_start(out=outd.ap(), in_=out_sb[:])

    nc.compile()
    return nc


def _prep_core(c, sentence_repr, esi, vidx, vmask, ht, dis_h, dis_t,
               dis_embed_b, w1_p, w2_p, b2_f):
    """Build the per-core input map for batches [c*NB, c*NB+NB)."""
    bs = range(c * NB, c * NB + NB)

    sent = np.empty((SENT_ROWS, D), dtype=BF16)
    for j, b in enumerate(bs):
        sent[j * S : (j + 1) * S] = sentence_repr[b].astype(BF16)
    sent[NEGROW:] = BF16(NEG)

    # quad-gather tables: batch h, pass j, span i = q*128+p (local);
    # pass 0 base = start (rows past w masked); pass 1 base = start+min(4, w-3),
    # valid only for w>=3 (else NEG quad)
    starts = np.stack([esi[b, :, 0] for b in bs])                 # (NB, NS)
    widths = np.stack([esi[b, :, 1] - esi[b, :, 0] for b in bs])  # (NB, NS)
    gidx = np.empty((128, NB, NPASS, 16), dtype=np.int16)
    for h in range(NB):
        st, w = starts[h], widths[h]
        for j in range(NPASS):
            if j == 0:
                idx = st + h * S
            else:
                idx = np.where(w >= 3, st + np.minimum(4, w - 3) + h * S, NEGROW)
            flat = idx.astype(np.int16)                           # i = q*128+p order
            gidx[:, h, j, :] = np.tile(flat.reshape(-1, 16).T, (8, 1))

    # additive mask for quad-0 rows 1..3: -2e30 where row r > width
    w0mv = np.zeros((128, NQ, 3), dtype=np.float32)
    wq = widths.reshape(NQ, 128).T                                 # [p, q]
    for r in (1, 2, 3):
        w0mv[:, :, r - 1] = np.where(wq < r, np.float32(-2e30), np.float32(0.0))

    poolt = np.zeros((128, NB, 2, V), dtype=BF16)
    invcnt = np.zeros((V, NB), dtype=np.float32)
    hsel = np.zeros((V, NB, R), dtype=BF16)
    tsel = np.zeros((V, NB, R), dtype=BF16)
    ehsel = np.zeros((DIS, NB, R), dtype=BF16)
    etsel = np.zeros((DIS, NB, R), dtype=BF16)
    for j, b in enumerate(bs):
        pt = np.zeros((NS, V), dtype=np.float32)
        np.add.at(pt, (vidx[b].ravel(), np.repeat(np.arange(V), C)), vmask[b].ravel().astype(np.float32))
        poolt[:, j] = pt.reshape(2, 128, V).transpose(1, 0, 2).astype(BF16)
        invcnt[:, j] = 1.0 / np.maximum(vmask[b].sum(axis=1).astype(np.float32), 1.0)
        hsel[ht[b, :, 0], j, np.arange(R)] = BF16(1.0)
        tsel[ht[b, :, 1], j, np.arange(R)] = BF16(1.0)
        ehsel[dis_h[b], j, np.arange(R)] = BF16(1.0)
        etsel[dis_t[b], j, np.arange(R)] = BF16(1.0)

    return dict(
        sent=sent, gidx=gidx, w0m=w0mv, poolt=poolt, invcnt=invcnt,
        hsel=hsel, tsel=tsel, dist=dis_embed_b.T.copy(), ehsel=ehsel, etsel=etsel,
        w1=w1_p, w2=w2_p, b2t=b2_f,
    )


def run(trace=False, **inputs):
    global _NC_CACHE
    sentence_repr = np.asarray(inputs["sentence_repr"], dtype=np.float32)
    esi = np.asarray(inputs["entity_span_indices"]).astype(np.int64)
    vidx = np.asarray(inputs["vertex_indices"]).astype(np.int64)
    vmask = np.asarray(inputs["vertex_indices_mask"]).astype(np.int64)
    ht = np.asarray(inputs["head_tail_indices"]).astype(np.int64)
    dis_h = np.asarray(inputs["dis_h_2_t"]).astype(np.int64)
    dis_t = np.asarray(inputs["dis_t_2_h"]).astype(np.int64)
    dis_embed = np.asarray(inputs["dis_embed"], dtype=np.float32)
    w1 = np.asarray(inputs["W1"], dtype=np.float32)
    w2 = np.asarray(inputs["W2"], dtype=np.float32)
    b2 = np.asarray(inputs["b2"], dtype=np.float32)

    dis_embed_b = dis_embed.astype(BF16)
    # zero-pad W1 blocks to 20 uniform 128-row chunks, laid out [p, chunk, :]
    w1_pad = np.zeros((W1PAD, HID), dtype=BF16)
    dst = 0
    for r0, r1 in FEAT_BLOCKS:
        rows = r1 - r0
        nch = (rows + 127) // 128
        for i in range(nch):
            a = r0 + i * 128
            n = min(128, r1 - a)
            w1_pad[dst : dst + n] = w1[a : a + n].astype(BF16)
            dst += 128
    assert dst == W1PAD
    w1_p = np.ascontiguousarray(w1_pad.reshape(NKC, 128, HID).transpose(1, 0, 2))
    w2_p = np.ascontiguousarray(w2.astype(BF16).reshape(HID // 128, 128, REL).transpose(1, 0, 2))
    b2_f = b2.reshape(REL, 1).astype(np.float32)

    in_maps = [
        _prep_core(c, sentence_repr, esi, vidx, vmask, ht, dis_h, dis_t,
                   dis_embed_b, w1_p, w2_p, b2_f)
        for c in range(NCORES)
    ]

    if _NC_CACHE is None:
        _NC_CACHE = _build()

    res = bass_utils.run_bass_kernel_spmd(
        _NC_CACHE, in_maps, core_ids=list(range(NCORES)), trace=trace
    )

    out = np.empty((B, R, REL), dtype=np.float32)
    for c in range(NCORES):
        o = np.asarray(res.results[c]["outd"], dtype=np.float32)  # (128, NB, R)
        for j in range(NB):
            out[c * NB + j] = o[:REL, j].T
    return out, res


def kernel(**inputs):
    out, _ = run(**inputs)
    return out
